# revision 51
# baseline (speedup 1.0000x reference)
"""Multi-head attention (B=2, S=2048, d_model=768, H=12) on 8 TRN2 NeuronCores.

Sharding: 2-way data parallel over batch x 4-way tensor parallel over heads
(3 heads / 192-wide d_model slice per core). Host compacts masked keys away
(gather of unmasked key/value rows), pads to a 128 multiple, and passes a 0/1
validity vector; softmax needs no mask handling on device (pad keys get V=0
and a 0 in the denominator ones-column). Per core:

    Q^T [192,2048], K^T [192,KP] via projections (dq on partitions)
    V   [KP,192] natural layout, x3 per-head [V_h | valid] blocks
    per head: scores^T[k,q] = K_h^T.T @ Q_h^T ; es = exp(s/8) on ACT
              ctx'^T[65,q] += [V_h|valid].T @ es  (row 64 = denominator)
              ctx = ctx * recip(denom) (DVE + gpsimd partition_broadcast)
    out_partial[2048,768] = ctx^T.T @ Wo_g, summed on host + bo.

All matmul operands are bf16 (PSUM accumulation f32); heads 0/1 issue score
matmuls into disjoint PE row groups back-to-back (2x concurrency) and share
one PSUM scores tile so a single ACT exp covers both heads. The output
projection is interleaved per query chunk to keep the PE warm.
"""

import math
import os

import numpy as np

B = 2
S = 2048
DM = 768
H = 12
DH = 64
G = 4              # head-group (tensor-parallel) degree
HPG = H // G       # heads per core
DQ = HPG * DH      # 192 d_model slice per core
NCORES = 8
P = 128

_prog_cache = {}


def _chunks(total, step):
    out = []
    o = 0
    while o < total:
        w = min(step, total - o)
        out.append((o, w))
        o += w
    return out


def _groups(n, g):
    out = []
    o = 0
    while o < n:
        out.append(list(range(o, min(o + g, n))))
        o += g
    return out


def _build_nc(KP):
    import concourse.bass as bass
    import concourse.mybir as mybir
    import concourse.tile as tile
    from concourse import bacc

    F32 = mybir.dt.float32
    BF = mybir.dt.bfloat16
    AFT = mybir.ActivationFunctionType

    T = KP // P            # key tiles
    NKT = DM // P          # 6 contraction tiles for projections
    QCH = _chunks(S, 512)
    KCH = _chunks(KP, 512)
    NCH = _chunks(DM, 512)

    nc = bacc.Bacc(None, target_bir_lowering=False)
    xqT = nc.declare_dram_parameter("xqT", [DM, S], BF, isOutput=False)
    xkT = nc.declare_dram_parameter("xkT", [DM, KP], BF, isOutput=False)
    xvT = nc.declare_dram_parameter("xvT", [DM, KP], BF, isOutput=False)
    wq = nc.declare_dram_parameter("wq", [DM, DQ], BF, isOutput=False)
    wk = nc.declare_dram_parameter("wk", [DM, DQ], BF, isOutput=False)
    wv = nc.declare_dram_parameter("wv", [DM, DQ], BF, isOutput=False)
    wo = nc.declare_dram_parameter("wo", [DQ, DM], BF, isOutput=False)
    bq = nc.declare_dram_parameter("bq", [DQ, 1], F32, isOutput=False)
    bk = nc.declare_dram_parameter("bk", [DQ, 1], F32, isOutput=False)
    bv = nc.declare_dram_parameter("bv", [1, DQ], F32, isOutput=False)
    vm = nc.declare_dram_parameter("vm", [P, T], F32, isOutput=False)
    out = nc.declare_dram_parameter("out", [S, DM], F32, isOutput=True)

    with tile.TileContext(nc) as tc:
        with (
            tc.tile_pool(name="persist", bufs=1) as persist,
            tc.tile_pool(name="acts", bufs=18) as acts,
            tc.tile_pool(name="es", bufs=3) as espool,
            tc.tile_pool(name="norm", bufs=4) as norm,
            tc.tile_pool(name="osb", bufs=3) as osb,
        ):
            # ---- weights / constants ----
            WQ = persist.tile([P, NKT, DQ], BF, tag="WQ")
            WK = persist.tile([P, NKT, DQ], BF, tag="WK")
            WV = persist.tile([P, NKT, DQ], BF, tag="WV")
            nc.sync.dma_start(out=WQ, in_=wq[:, :].rearrange("(kt p) m -> p kt m", p=P))
            nc.sync.dma_start(out=WK, in_=wk[:, :].rearrange("(kt p) m -> p kt m", p=P))
            nc.sync.dma_start(out=WV, in_=wv[:, :].rearrange("(kt p) m -> p kt m", p=P))
            WO0 = persist.tile([P, DM], BF, tag="WO0")   # wo rows 0:128 (h0,h1)
            WO2 = persist.tile([DH, DM], BF, tag="WO2")  # wo rows 128:192 (h2)
            nc.sync.dma_start(out=WO0, in_=wo[0:P, :])
            nc.sync.dma_start(out=WO2, in_=wo[P:DQ, :])
            BQ0 = persist.tile([P, 1], F32, tag="BQ0")
            BQ1 = persist.tile([DH, 1], F32, tag="BQ1")
            BK0 = persist.tile([P, 1], F32, tag="BK0")
            BK1 = persist.tile([DH, 1], F32, tag="BK1")
            nc.sync.dma_start(out=BQ0, in_=bq[0:P, :])
            nc.sync.dma_start(out=BQ1, in_=bq[P:DQ, :])
            nc.sync.dma_start(out=BK0, in_=bk[0:P, :])
            nc.sync.dma_start(out=BK1, in_=bk[P:DQ, :])
            BV = persist.tile([P, DQ], F32, tag="BV")
            nc.sync.dma_start(out=BV, in_=bv[:, :].to_broadcast([P, DQ]))
            VM = persist.tile([P, T], F32, tag="VM")
            nc.sync.dma_start(out=VM, in_=vm[:, :])

            # ---- persistent activations ----
            QT0 = persist.tile([P, S], BF, tag="QT0")    # heads 0,1
            QT1 = persist.tile([DH, S], BF, tag="QT1")   # head 2
            KT0 = persist.tile([P, KP], BF, tag="KT0")
            KT1 = persist.tile([DH, KP], BF, tag="KT1")
            # V blocks padded to 128 cols (cols 0:64 V, 64 ones, 65:128 zero)
            # so PV ldweights gets FWL (needs exactly 128 weight columns)
            VP = persist.tile([P, T, HPG * P], BF, tag="VP")
            nc.vector.memset(VP, 0.0)
            CTX01 = persist.tile([P, S], BF, tag="CTX01")  # h0 rows 0:64, h1 64:128
            CTX2 = persist.tile([DH, S], BF, tag="CTX2")

            # One PSUM pool set for the whole kernel (no pool-scope barriers):
            # big: 2-bank slots x2 (scores double-buffer + O-proj psum)
            # sml: 1-bank slots x3 (projection psums + ctx accumulators)
            _big_cm = tc.tile_pool(name="big_ps", bufs=2, space="PSUM")
            _sml_cm = tc.tile_pool(name="sml_ps", bufs=2, space="PSUM")
            _opo_cm = tc.tile_pool(name="opo_ps", bufs=1, space="PSUM")
            big_ps = _big_cm.__enter__()
            sml_ps = _sml_cm.__enter__()
            opo_ps = _opo_cm.__enter__()
            ctx_stack = [_big_cm, _sml_cm, _opo_cm]

            # ---- phase A: projections (KT -> QT -> VP) ----
            XK = []
            for kt in range(NKT):
                xt = acts.tile([P, S], BF, tag="xact", name=f"xk{kt}")
                nc.sync.dma_start(out=xt[:, 0:KP], in_=xkT[kt * P:(kt + 1) * P, :])
                XK.append(xt)
            for (c0, cw) in KCH:
                for m, (dst, bias, mw) in enumerate(
                    [(KT0, BK0, P), (KT1, BK1, DH)]
                ):
                    ps = sml_ps.tile([mw, 512], F32, tag="sml", name=f"kps{c0}_{m}")
                    for kt in range(NKT):
                        nc.tensor.matmul(
                            ps[:, 0:cw],
                            lhsT=WK[:, kt, m * P:m * P + mw],
                            rhs=XK[kt][:, c0:c0 + cw],
                            start=(kt == 0),
                            stop=(kt == NKT - 1),
                        )
                    nc.scalar.activation(
                        dst[:, c0:c0 + cw], ps[:, 0:cw], AFT.Identity, bias=bias
                    )
            XQ = []
            for kt in range(NKT):
                xt = acts.tile([P, S], BF, tag="xact", name=f"xq{kt}")
                nc.sync.dma_start(out=xt, in_=xqT[kt * P:(kt + 1) * P, :])
                XQ.append(xt)
            for (c0, cw) in QCH:
                for m, (dst, bias, mw) in enumerate(
                    [(QT0, BQ0, P), (QT1, BQ1, DH)]
                ):
                    ps = sml_ps.tile([mw, 512], F32, tag="sml", name=f"qps{c0}_{m}")
                    for kt in range(NKT):
                        nc.tensor.matmul(
                            ps[:, 0:cw],
                            lhsT=WQ[:, kt, m * P:m * P + mw],
                            rhs=XQ[kt][:, c0:c0 + cw],
                            start=(kt == 0),
                            stop=(kt == NKT - 1),
                        )
                    nc.scalar.activation(
                        dst[:, c0:c0 + cw], ps[:, 0:cw], AFT.Identity, bias=bias
                    )
            XV = []
            for kt in range(NKT):
                xt = acts.tile([P, S], BF, tag="xact", name=f"xv{kt}")
                nc.sync.dma_start(out=xt[:, 0:KP], in_=xvT[kt * P:(kt + 1) * P, :])
                XV.append(xt)
            for t in range(T):
                ps = sml_ps.tile([P, DQ], F32, tag="sml", name=f"vps{t}")
                for kt in range(NKT):
                    nc.tensor.matmul(
                        ps,
                        lhsT=XV[kt][:, t * P:(t + 1) * P],
                        rhs=WV[:, kt, :],
                        start=(kt == 0),
                        stop=(kt == NKT - 1),
                    )
                vview = VP[:, t, :].rearrange("p (h c) -> p h c", c=P)
                nc.vector.tensor_add(
                    vview[:, :, 0:DH],
                    ps.rearrange("p (h d) -> p h d", d=DH),
                    BV[:, :].rearrange("p (h d) -> p h d", d=DH),
                )
                nc.vector.tensor_scalar_mul(
                    vview[:, :, 0:DH], vview[:, :, 0:DH], VM[:, t:t + 1]
                )
                nc.vector.tensor_copy(
                    vview[:, :, DH:DH + 1],
                    VM[:, t:t + 1].to_broadcast([P, HPG, 1]),
                )

            # ---- phase B+C: attention + output projection, per query chunk ----
            def attn_norm(ctx, dst, c0, cw, uid):
                # evict PSUM on ACT (frees the ctx bank + off DVE critical
                # path), then normalize from SBUF
                cs = norm.tile([DH, 512], F32, tag="cs", name=f"cs{uid}")
                nc.scalar.activation(cs[:, 0:cw], ctx[0:DH, 0:cw], AFT.Identity)
                dn = norm.tile([1, 512], F32, tag="dn", name=f"dn{uid}")
                nc.vector.tensor_copy(dn[:, 0:cw], ctx[DH:DH + 1, 0:cw])
                rc = norm.tile([1, 512], F32, tag="rc", name=f"rc{uid}")
                nc.vector.reciprocal_approx_fast(rc[:, 0:cw], dn[:, 0:cw])
                bc = norm.tile([DH, 512], F32, tag="bc", name=f"bc{uid}")
                nc.gpsimd.partition_broadcast(bc[:, 0:cw], rc[:, 0:cw])
                nc.vector.tensor_mul(dst[:, c0:c0 + cw], cs[:, 0:cw], bc[:, 0:cw])

            for ci, (c0, cw) in enumerate(QCH):
                # heads 0+1 paired: adjacent matmuls into disjoint PE row
                # groups (2x concurrency), one shared scores tile + exp per t;
                # scores double-buffered so PE streams through ACT exps.
                ctx0 = sml_ps.tile([P, 512], F32, tag="sml", name=f"c0_{ci}")
                ctx1 = sml_ps.tile([P, 512], F32, tag="sml", name=f"c1_{ci}")
                for t in range(T):
                    sp = big_ps.tile([P, 2 * 512], F32, tag="big", name=f"sp{ci}_{t}")
                    nc.tensor.matmul(
                        sp[:, 0:cw],
                        lhsT=KT0[0:DH, t * P:(t + 1) * P],
                        rhs=QT0[0:DH, c0:c0 + cw],
                        start=True, stop=True,
                    )
                    nc.tensor.matmul(
                        sp[:, 512:512 + cw],
                        lhsT=KT0[DH:P, t * P:(t + 1) * P],
                        rhs=QT0[DH:P, c0:c0 + cw],
                        start=True, stop=True,
                    )
                    es = espool.tile([P, 2 * 512], BF, tag="es", name=f"es{ci}_{t}")
                    nc.scalar.activation(
                        es, sp, AFT.Exp, bias=0.0, scale=1.0 / math.sqrt(DH),
                    )
                    nc.tensor.matmul(
                        ctx0[:, 0:cw],
                        lhsT=VP[:, t, 0:P],
                        rhs=es[:, 0:cw],
                        start=(t == 0), stop=(t == T - 1),
                    )
                    nc.tensor.matmul(
                        ctx1[:, 0:cw],
                        lhsT=VP[:, t, P:2 * P],
                        rhs=es[:, 512:512 + cw],
                        start=(t == 0), stop=(t == T - 1),
                    )
                attn_norm(ctx0, CTX01[0:DH, :], c0, cw, f"a{ci}")
                attn_norm(ctx1, CTX01[DH:P, :], c0, cw, f"b{ci}")
                # head 2: two key tiles per scores buffer
                ctx2 = sml_ps.tile([P, 512], F32, tag="sml", name=f"c2_{ci}")
                for tg in _groups(T, 2):
                    ln = len(tg)
                    sp = big_ps.tile([P, 2 * 512], F32, tag="big", name=f"sp2_{ci}_{tg[0]}")
                    for i, t in enumerate(tg):
                        nc.tensor.matmul(
                            sp[:, i * 512:i * 512 + cw],
                            lhsT=KT1[0:DH, t * P:(t + 1) * P],
                            rhs=QT1[0:DH, c0:c0 + cw],
                            start=True, stop=True,
                        )
                    es = espool.tile([P, 2 * 512], BF, tag="es", name=f"es2_{ci}_{tg[0]}")
                    nc.scalar.activation(
                        es[:, 0:ln * 512], sp[:, 0:ln * 512],
                        AFT.Exp, bias=0.0, scale=1.0 / math.sqrt(DH),
                    )
                    for i, t in enumerate(tg):
                        nc.tensor.matmul(
                            ctx2[:, 0:cw],
                            lhsT=VP[:, t, 2 * P:3 * P],
                            rhs=es[:, i * 512:i * 512 + cw],
                            start=(t == 0), stop=(t == T - 1),
                        )
                attn_norm(ctx2, CTX2, c0, cw, f"c{ci}")
                # output projection for this chunk's query tiles
                for mi in range(cw // P):
                    m = c0 // P + mi
                    po = opo_ps.tile([P, DM], F32, tag="opo", name=f"po{m}")
                    for (n0, nw) in NCH:
                        nc.tensor.matmul(
                            po[:, n0:n0 + nw],
                            lhsT=CTX01[:, m * P:(m + 1) * P],
                            rhs=WO0[:, n0:n0 + nw],
                            start=True, stop=False,
                        )
                        nc.tensor.matmul(
                            po[:, n0:n0 + nw],
                            lhsT=CTX2[:, m * P:(m + 1) * P],
                            rhs=WO2[:, n0:n0 + nw],
                            start=False, stop=True,
                        )
                    po_sb = osb.tile([P, DM], F32, tag="posb", name=f"posb{m}")
                    nc.vector.tensor_copy(po_sb, po)
                    nc.sync.dma_start(out=out[m * P:(m + 1) * P, :], in_=po_sb)

            for _p in reversed(ctx_stack):
                _p.__exit__(None, None, None)
    nc.compile()
    return nc


def _get_prog(KP):
    if KP not in _prog_cache:
        _prog_cache[KP] = _build_nc(KP)
    return _prog_cache[KP]


def _run(inputs, trace=False):
    import ml_dtypes
    from concourse.bass_utils import run_bass_kernel_spmd

    BF = ml_dtypes.bfloat16

    query = np.asarray(inputs["query"], dtype=np.float32)
    key = np.asarray(inputs["key"], dtype=np.float32)
    value = np.asarray(inputs["value"], dtype=np.float32)
    mask = np.asarray(inputs["mask"])
    Wq = np.asarray(inputs["Wq"], dtype=np.float32)
    bq = np.asarray(inputs["bq"], dtype=np.float32)
    Wk = np.asarray(inputs["Wk"], dtype=np.float32)
    bk = np.asarray(inputs["bk"], dtype=np.float32)
    Wv = np.asarray(inputs["Wv"], dtype=np.float32)
    bv = np.asarray(inputs["bv"], dtype=np.float32)
    Wo = np.asarray(inputs["Wo"], dtype=np.float32)
    bo = np.asarray(inputs["bo"], dtype=np.float32)

    idx = [np.nonzero(mask[b, 0, 0] != 0)[0] for b in range(B)]
    keff = [len(i) for i in idx]
    KP = max(P, ((max(keff) + P - 1) // P) * P)
    T = KP // P

    nc = _get_prog(KP)

    per_batch = {}
    for b in range(B):
        xqT = np.ascontiguousarray(query[b].T).astype(BF)
        xkT = np.zeros((DM, KP), dtype=BF)
        xkT[:, :keff[b]] = key[b][idx[b]].T.astype(BF)
        xvT = np.zeros((DM, KP), dtype=BF)
        xvT[:, :keff[b]] = value[b][idx[b]].T.astype(BF)
        vmf = np.zeros((KP,), dtype=np.float32)
        vmf[:keff[b]] = 1.0
        vm2 = np.ascontiguousarray(vmf.reshape(T, P).T)  # [128, T]
        per_batch[b] = (xqT, xkT, xvT, vm2)

    in_maps = []
    for core in range(NCORES):
        b, g = core // G, core % G
        xqT, xkT, xvT, vm2 = per_batch[b]
        sl = slice(g * DQ, (g + 1) * DQ)
        in_maps.append({
            "xqT": xqT,
            "xkT": xkT,
            "xvT": xvT,
            "wq": np.ascontiguousarray(Wq[:, sl]).astype(BF),
            "wk": np.ascontiguousarray(Wk[:, sl]).astype(BF),
            "wv": np.ascontiguousarray(Wv[:, sl]).astype(BF),
            "wo": np.ascontiguousarray(Wo[sl, :]).astype(BF),
            "bq": np.ascontiguousarray(bq[sl].reshape(DQ, 1)),
            "bk": np.ascontiguousarray(bk[sl].reshape(DQ, 1)),
            "bv": np.ascontiguousarray(bv[sl].reshape(1, DQ)),
            "vm": vm2,
        })

    res = run_bass_kernel_spmd(nc, in_maps, list(range(NCORES)), trace=trace)

    outp = np.zeros((B, S, DM), dtype=np.float32)
    for core in range(NCORES):
        outp[core // G] += res.results[core]["out"]
    outp += bo.reshape(1, 1, DM)
    return outp, res


def kernel(**inputs) -> np.ndarray:
    out, _ = _run(inputs, trace=False)
    return out


if __name__ == "__main__":
    nc = _build_nc(1152)
    print("build OK")


# revision 52
# speedup vs baseline: 1.0390x; 1.0390x over previous
"""Multi-head attention (B=2, S=2048, d_model=768, H=12) on 8 TRN2 NeuronCores.

Sharding: 2-way data parallel over batch x 4-way tensor parallel over heads
(3 heads / 192-wide d_model slice per core). Host compacts masked keys away
(gather of unmasked key/value rows), pads to a 128 multiple, and passes a 0/1
validity vector; softmax needs no mask handling on device (pad keys get V=0
and a 0 in the denominator ones-column). Per core:

    Q^T [192,2048], K^T [192,KP] via projections (dq on partitions)
    V   [KP,192] natural layout, x3 per-head [V_h | valid] blocks
    per head: scores^T[k,q] = K_h^T.T @ Q_h^T ; es = exp(s/8) on ACT
              ctx'^T[65,q] += [V_h|valid].T @ es  (row 64 = denominator)
              ctx = ctx * recip(denom) (DVE + gpsimd partition_broadcast)
    out_partial[2048,768] = ctx^T.T @ Wo_g, summed on host + bo.

All matmul operands are bf16 (PSUM accumulation f32); heads 0/1 issue score
matmuls into disjoint PE row groups back-to-back (2x concurrency) and share
one PSUM scores tile so a single ACT exp covers both heads. The output
projection is interleaved per query chunk to keep the PE warm.
"""

import math
import os

import numpy as np

B = 2
S = 2048
DM = 768
H = 12
DH = 64
G = 4              # head-group (tensor-parallel) degree
HPG = H // G       # heads per core
DQ = HPG * DH      # 192 d_model slice per core
NCORES = 8
P = 128

_prog_cache = {}


def _chunks(total, step):
    out = []
    o = 0
    while o < total:
        w = min(step, total - o)
        out.append((o, w))
        o += w
    return out


def _groups(n, g):
    out = []
    o = 0
    while o < n:
        out.append(list(range(o, min(o + g, n))))
        o += g
    return out


def _build_nc(KP):
    import concourse.bass as bass
    import concourse.mybir as mybir
    import concourse.tile as tile
    from concourse import bacc

    F32 = mybir.dt.float32
    BF = mybir.dt.bfloat16
    AFT = mybir.ActivationFunctionType

    T = KP // P            # key tiles
    NKT = DM // P          # 6 contraction tiles for projections
    QCH = _chunks(S, 512)
    KCH = _chunks(KP, 512)
    NCH = _chunks(DM, 512)

    nc = bacc.Bacc(None, target_bir_lowering=False)
    xqT = nc.declare_dram_parameter("xqT", [DM, S], BF, isOutput=False)
    xkT = nc.declare_dram_parameter("xkT", [DM, KP], BF, isOutput=False)
    xvT = nc.declare_dram_parameter("xvT", [DM, KP], BF, isOutput=False)
    wq = nc.declare_dram_parameter("wq", [DM, DQ], BF, isOutput=False)
    wk = nc.declare_dram_parameter("wk", [DM, DQ], BF, isOutput=False)
    wv = nc.declare_dram_parameter("wv", [DM, DQ], BF, isOutput=False)
    wo = nc.declare_dram_parameter("wo", [DQ, DM], BF, isOutput=False)
    bq = nc.declare_dram_parameter("bq", [DQ, 1], F32, isOutput=False)
    bk = nc.declare_dram_parameter("bk", [DQ, 1], F32, isOutput=False)
    bv = nc.declare_dram_parameter("bv", [1, DQ], F32, isOutput=False)
    vm = nc.declare_dram_parameter("vm", [P, T], F32, isOutput=False)
    out = nc.declare_dram_parameter("out", [S, DM], F32, isOutput=True)

    with tile.TileContext(nc) as tc:
        with (
            tc.tile_pool(name="persist", bufs=1) as persist,
            tc.tile_pool(name="acts", bufs=18) as acts,
            tc.tile_pool(name="es", bufs=4) as espool,
            tc.tile_pool(name="norm", bufs=6) as norm,
            tc.tile_pool(name="osb", bufs=4) as osb,
        ):
            # ---- weights / constants ----
            WQ = persist.tile([P, NKT, DQ], BF, tag="WQ")
            WK = persist.tile([P, NKT, DQ], BF, tag="WK")
            WV = persist.tile([P, NKT, DQ], BF, tag="WV")
            nc.sync.dma_start(out=WQ, in_=wq[:, :].rearrange("(kt p) m -> p kt m", p=P))
            nc.sync.dma_start(out=WK, in_=wk[:, :].rearrange("(kt p) m -> p kt m", p=P))
            nc.sync.dma_start(out=WV, in_=wv[:, :].rearrange("(kt p) m -> p kt m", p=P))
            WO0 = persist.tile([P, DM], BF, tag="WO0")   # wo rows 0:128 (h0,h1)
            WO2 = persist.tile([DH, DM], BF, tag="WO2")  # wo rows 128:192 (h2)
            nc.sync.dma_start(out=WO0, in_=wo[0:P, :])
            nc.sync.dma_start(out=WO2, in_=wo[P:DQ, :])
            BQ0 = persist.tile([P, 1], F32, tag="BQ0")
            BQ1 = persist.tile([DH, 1], F32, tag="BQ1")
            BK0 = persist.tile([P, 1], F32, tag="BK0")
            BK1 = persist.tile([DH, 1], F32, tag="BK1")
            nc.sync.dma_start(out=BQ0, in_=bq[0:P, :])
            nc.sync.dma_start(out=BQ1, in_=bq[P:DQ, :])
            nc.sync.dma_start(out=BK0, in_=bk[0:P, :])
            nc.sync.dma_start(out=BK1, in_=bk[P:DQ, :])
            BV = persist.tile([P, DQ], F32, tag="BV")
            nc.sync.dma_start(out=BV, in_=bv[:, :].to_broadcast([P, DQ]))
            VM = persist.tile([P, T], F32, tag="VM")
            nc.sync.dma_start(out=VM, in_=vm[:, :])

            # ---- persistent activations ----
            QT0 = persist.tile([P, S], BF, tag="QT0")    # heads 0,1
            QT1 = persist.tile([DH, S], BF, tag="QT1")   # head 2
            KT0 = persist.tile([P, KP], BF, tag="KT0")
            KT1 = persist.tile([DH, KP], BF, tag="KT1")
            # V blocks padded to 128 cols (cols 0:64 V, 64 ones, 65:128 zero)
            # so PV ldweights gets FWL (needs exactly 128 weight columns)
            VP = persist.tile([P, T, HPG * P], BF, tag="VP")
            nc.vector.memset(VP, 0.0)
            CTX01 = persist.tile([P, S], BF, tag="CTX01")  # h0 rows 0:64, h1 64:128
            CTX2 = persist.tile([DH, S], BF, tag="CTX2")

            # One PSUM pool set for the whole kernel (no pool-scope barriers):
            # big: 2-bank slots x2 (scores double-buffer + O-proj psum)
            # sml: 1-bank slots x3 (projection psums + ctx accumulators)
            _big_cm = tc.tile_pool(name="big_ps", bufs=2, space="PSUM")
            _sml_cm = tc.tile_pool(name="sml_ps", bufs=2, space="PSUM")
            _opo_cm = tc.tile_pool(name="opo_ps", bufs=1, space="PSUM")
            big_ps = _big_cm.__enter__()
            sml_ps = _sml_cm.__enter__()
            opo_ps = _opo_cm.__enter__()
            ctx_stack = [_big_cm, _sml_cm, _opo_cm]

            # ---- phase A: projections (KT -> QT -> VP) ----
            XK = []
            for kt in range(NKT):
                xt = acts.tile([P, S], BF, tag="xact", name=f"xk{kt}")
                nc.sync.dma_start(out=xt[:, 0:KP], in_=xkT[kt * P:(kt + 1) * P, :])
                XK.append(xt)
            for (c0, cw) in KCH:
                for m, (dst, bias, mw) in enumerate(
                    [(KT0, BK0, P), (KT1, BK1, DH)]
                ):
                    ps = sml_ps.tile([mw, 512], F32, tag="sml", name=f"kps{c0}_{m}")
                    for kt in range(NKT):
                        nc.tensor.matmul(
                            ps[:, 0:cw],
                            lhsT=WK[:, kt, m * P:m * P + mw],
                            rhs=XK[kt][:, c0:c0 + cw],
                            start=(kt == 0),
                            stop=(kt == NKT - 1),
                        )
                    nc.scalar.activation(
                        dst[:, c0:c0 + cw], ps[:, 0:cw], AFT.Identity, bias=bias
                    )
            XQ = []
            for kt in range(NKT):
                xt = acts.tile([P, S], BF, tag="xact", name=f"xq{kt}")
                nc.sync.dma_start(out=xt, in_=xqT[kt * P:(kt + 1) * P, :])
                XQ.append(xt)
            for (c0, cw) in QCH:
                for m, (dst, bias, mw) in enumerate(
                    [(QT0, BQ0, P), (QT1, BQ1, DH)]
                ):
                    ps = sml_ps.tile([mw, 512], F32, tag="sml", name=f"qps{c0}_{m}")
                    for kt in range(NKT):
                        nc.tensor.matmul(
                            ps[:, 0:cw],
                            lhsT=WQ[:, kt, m * P:m * P + mw],
                            rhs=XQ[kt][:, c0:c0 + cw],
                            start=(kt == 0),
                            stop=(kt == NKT - 1),
                        )
                    nc.scalar.activation(
                        dst[:, c0:c0 + cw], ps[:, 0:cw], AFT.Identity, bias=bias
                    )
            XV = []
            for kt in range(NKT):
                xt = acts.tile([P, S], BF, tag="xact", name=f"xv{kt}")
                nc.sync.dma_start(out=xt[:, 0:KP], in_=xvT[kt * P:(kt + 1) * P, :])
                XV.append(xt)
            for t in range(T):
                ps = sml_ps.tile([P, DQ], F32, tag="sml", name=f"vps{t}")
                for kt in range(NKT):
                    nc.tensor.matmul(
                        ps,
                        lhsT=XV[kt][:, t * P:(t + 1) * P],
                        rhs=WV[:, kt, :],
                        start=(kt == 0),
                        stop=(kt == NKT - 1),
                    )
                vview = VP[:, t, :].rearrange("p (h c) -> p h c", c=P)
                nc.vector.tensor_add(
                    vview[:, :, 0:DH],
                    ps.rearrange("p (h d) -> p h d", d=DH),
                    BV[:, :].rearrange("p (h d) -> p h d", d=DH),
                )
                nc.vector.tensor_scalar_mul(
                    vview[:, :, 0:DH], vview[:, :, 0:DH], VM[:, t:t + 1]
                )
                nc.vector.tensor_copy(
                    vview[:, :, DH:DH + 1],
                    VM[:, t:t + 1].to_broadcast([P, HPG, 1]),
                )

            # ---- phase B+C: attention + output projection, per query chunk ----
            def attn_norm(ctx, dst, c0, cw, uid):
                # evict PSUM on ACT (frees the ctx bank + off DVE critical
                # path), then normalize from SBUF
                cs = norm.tile([DH, 512], F32, tag="cs", name=f"cs{uid}")
                nc.scalar.activation(cs[:, 0:cw], ctx[0:DH, 0:cw], AFT.Identity)
                dn = norm.tile([1, 512], F32, tag="dn", name=f"dn{uid}")
                nc.vector.tensor_copy(dn[:, 0:cw], ctx[DH:DH + 1, 0:cw])
                rc = norm.tile([1, 512], F32, tag="rc", name=f"rc{uid}")
                nc.vector.reciprocal_approx_fast(rc[:, 0:cw], dn[:, 0:cw])
                bc = norm.tile([DH, 512], F32, tag="bc", name=f"bc{uid}")
                nc.gpsimd.partition_broadcast(bc[:, 0:cw], rc[:, 0:cw])
                nc.vector.tensor_mul(dst[:, c0:c0 + cw], cs[:, 0:cw], bc[:, 0:cw])

            for ci, (c0, cw) in enumerate(QCH):
                # heads 0+1 paired: adjacent matmuls into disjoint PE row
                # groups (2x concurrency), one shared scores tile + exp per t;
                # scores double-buffered so PE streams through ACT exps.
                ctx0 = sml_ps.tile([P, 512], F32, tag="sml", name=f"c0_{ci}")
                ctx1 = sml_ps.tile([P, 512], F32, tag="sml", name=f"c1_{ci}")
                for t in range(T):
                    sp = big_ps.tile([P, 2 * 512], F32, tag="big", name=f"sp{ci}_{t}")
                    nc.tensor.matmul(
                        sp[:, 0:cw],
                        lhsT=KT0[0:DH, t * P:(t + 1) * P],
                        rhs=QT0[0:DH, c0:c0 + cw],
                        start=True, stop=True,
                    )
                    nc.tensor.matmul(
                        sp[:, 512:512 + cw],
                        lhsT=KT0[DH:P, t * P:(t + 1) * P],
                        rhs=QT0[DH:P, c0:c0 + cw],
                        start=True, stop=True,
                    )
                    es = espool.tile([P, 2 * 512], BF, tag="es", name=f"es{ci}_{t}")
                    nc.scalar.activation(
                        es, sp, AFT.Exp, bias=0.0, scale=1.0 / math.sqrt(DH),
                    )
                    nc.tensor.matmul(
                        ctx0[:, 0:cw],
                        lhsT=VP[:, t, 0:P],
                        rhs=es[:, 0:cw],
                        start=(t == 0), stop=(t == T - 1),
                    )
                    nc.tensor.matmul(
                        ctx1[:, 0:cw],
                        lhsT=VP[:, t, P:2 * P],
                        rhs=es[:, 512:512 + cw],
                        start=(t == 0), stop=(t == T - 1),
                    )
                attn_norm(ctx0, CTX01[0:DH, :], c0, cw, f"a{ci}")
                attn_norm(ctx1, CTX01[DH:P, :], c0, cw, f"b{ci}")
                # head 2: two key tiles per scores buffer
                ctx2 = sml_ps.tile([P, 512], F32, tag="sml", name=f"c2_{ci}")
                for tg in _groups(T, 2):
                    ln = len(tg)
                    sp = big_ps.tile([P, 2 * 512], F32, tag="big", name=f"sp2_{ci}_{tg[0]}")
                    for i, t in enumerate(tg):
                        nc.tensor.matmul(
                            sp[:, i * 512:i * 512 + cw],
                            lhsT=KT1[0:DH, t * P:(t + 1) * P],
                            rhs=QT1[0:DH, c0:c0 + cw],
                            start=True, stop=True,
                        )
                    es = espool.tile([P, 2 * 512], BF, tag="es", name=f"es2_{ci}_{tg[0]}")
                    nc.scalar.activation(
                        es[:, 0:ln * 512], sp[:, 0:ln * 512],
                        AFT.Exp, bias=0.0, scale=1.0 / math.sqrt(DH),
                    )
                    for i, t in enumerate(tg):
                        nc.tensor.matmul(
                            ctx2[:, 0:cw],
                            lhsT=VP[:, t, 2 * P:3 * P],
                            rhs=es[:, i * 512:i * 512 + cw],
                            start=(t == 0), stop=(t == T - 1),
                        )
                attn_norm(ctx2, CTX2, c0, cw, f"c{ci}")
                # output projection for this chunk's query tiles
                for mi in range(cw // P):
                    m = c0 // P + mi
                    po = opo_ps.tile([P, DM], F32, tag="opo", name=f"po{m}")
                    for (n0, nw) in NCH:
                        nc.tensor.matmul(
                            po[:, n0:n0 + nw],
                            lhsT=CTX01[:, m * P:(m + 1) * P],
                            rhs=WO0[:, n0:n0 + nw],
                            start=True, stop=False,
                        )
                        nc.tensor.matmul(
                            po[:, n0:n0 + nw],
                            lhsT=CTX2[:, m * P:(m + 1) * P],
                            rhs=WO2[:, n0:n0 + nw],
                            start=False, stop=True,
                        )
                    po_sb = osb.tile([P, DM], F32, tag="posb", name=f"posb{m}")
                    nc.vector.tensor_copy(po_sb, po)
                    nc.sync.dma_start(out=out[m * P:(m + 1) * P, :], in_=po_sb)

            for _p in reversed(ctx_stack):
                _p.__exit__(None, None, None)
    nc.compile()
    return nc


def _get_prog(KP):
    if KP not in _prog_cache:
        _prog_cache[KP] = _build_nc(KP)
    return _prog_cache[KP]


def _run(inputs, trace=False):
    import ml_dtypes
    from concourse.bass_utils import run_bass_kernel_spmd

    BF = ml_dtypes.bfloat16

    query = np.asarray(inputs["query"], dtype=np.float32)
    key = np.asarray(inputs["key"], dtype=np.float32)
    value = np.asarray(inputs["value"], dtype=np.float32)
    mask = np.asarray(inputs["mask"])
    Wq = np.asarray(inputs["Wq"], dtype=np.float32)
    bq = np.asarray(inputs["bq"], dtype=np.float32)
    Wk = np.asarray(inputs["Wk"], dtype=np.float32)
    bk = np.asarray(inputs["bk"], dtype=np.float32)
    Wv = np.asarray(inputs["Wv"], dtype=np.float32)
    bv = np.asarray(inputs["bv"], dtype=np.float32)
    Wo = np.asarray(inputs["Wo"], dtype=np.float32)
    bo = np.asarray(inputs["bo"], dtype=np.float32)

    idx = [np.nonzero(mask[b, 0, 0] != 0)[0] for b in range(B)]
    keff = [len(i) for i in idx]
    KP = max(P, ((max(keff) + P - 1) // P) * P)
    T = KP // P

    nc = _get_prog(KP)

    per_batch = {}
    for b in range(B):
        xqT = np.ascontiguousarray(query[b].T).astype(BF)
        xkT = np.zeros((DM, KP), dtype=BF)
        xkT[:, :keff[b]] = key[b][idx[b]].T.astype(BF)
        xvT = np.zeros((DM, KP), dtype=BF)
        xvT[:, :keff[b]] = value[b][idx[b]].T.astype(BF)
        vmf = np.zeros((KP,), dtype=np.float32)
        vmf[:keff[b]] = 1.0
        vm2 = np.ascontiguousarray(vmf.reshape(T, P).T)  # [128, T]
        per_batch[b] = (xqT, xkT, xvT, vm2)

    in_maps = []
    for core in range(NCORES):
        b, g = core // G, core % G
        xqT, xkT, xvT, vm2 = per_batch[b]
        sl = slice(g * DQ, (g + 1) * DQ)
        in_maps.append({
            "xqT": xqT,
            "xkT": xkT,
            "xvT": xvT,
            "wq": np.ascontiguousarray(Wq[:, sl]).astype(BF),
            "wk": np.ascontiguousarray(Wk[:, sl]).astype(BF),
            "wv": np.ascontiguousarray(Wv[:, sl]).astype(BF),
            "wo": np.ascontiguousarray(Wo[sl, :]).astype(BF),
            "bq": np.ascontiguousarray(bq[sl].reshape(DQ, 1)),
            "bk": np.ascontiguousarray(bk[sl].reshape(DQ, 1)),
            "bv": np.ascontiguousarray(bv[sl].reshape(1, DQ)),
            "vm": vm2,
        })

    res = run_bass_kernel_spmd(nc, in_maps, list(range(NCORES)), trace=trace)

    outp = np.zeros((B, S, DM), dtype=np.float32)
    for core in range(NCORES):
        outp[core // G] += res.results[core]["out"]
    outp += bo.reshape(1, 1, DM)
    return outp, res


def kernel(**inputs) -> np.ndarray:
    out, _ = _run(inputs, trace=False)
    return out


if __name__ == "__main__":
    nc = _build_nc(1152)
    print("build OK")
